# revision 1
# baseline (speedup 1.0000x reference)
"""CausalBiTrilinearBCNAttention Trainium2 kernel.

Math refactorization: every use of Q, K, invQ, invK in the reference is
through a rank-R projection, and causal cumsum commutes with right
multiplication, so the network collapses to

    xp  = x @ P                  P = [a1|a2s|a3|b1|b2|b3|b7]  (D x 448)
    cumc = causal_cumsum(xp[..., 192:448]) / counts
    g1  = xp[:,0:64]*cumc[:,0:64] + xp[:,64:128]*cumc[:,64:128]
    g2  = xp[:,128:192]*cumc[:,128:192] * cumc[:,192:256]
    out = [g1|g2] @ A.T          A = [WO@U_b | alpha_tri*WO@U_t]  (D x 128)

with  a1=WQ^T V_b, a2s=alpha_bi WQ^T Winv^T W_b, a3=WQ^T V_t,
      b1=WK^T W_b, b2=WK^T Winv^T V_b, b3=WK^T W_t, b7=X_t.

Sharding: 8 cores = 4 batches x 2 T-halves; the T/2 cumsum carry for the
second half is rebuilt on device from sx = sum_t x[b,:T/2] (host computes
only the data reduction; sx @ P happens on device).

Device dataflow, all [token-partition, feature-free], fp16 operands with
fp32 PSUM accumulation (~5e-4 relative error end to end):
  - xp matmuls stream dk-chunks right behind the interleaved xT/P DMAs
    so TensorE starts early and stays HAM-warm.
  - per-128-token-tile cumsum = U^T @ xp_tile on PE (U = upper-tri ones),
    carry broadcast added in the same PSUM group via Ek^T @ cumc_{k-1},
    Ek one-hot row 127 scaled by counts[last token of k-1] (cumc is
    stored pre-divided by counts; counts <= 2048 are exact in fp16).
  - G transposed 128x128 on PE, final [T,128]@[128,D] matmul in fp16.
"""

import numpy as np

import concourse.bass as bass
import concourse.tile as tile
from concourse import bacc, mybir
from concourse.bass_utils import run_bass_kernel_spmd

B, T, D, R = 4, 2048, 1024, 64
TH = T // 2          # tokens per core
NT = TH // 128       # 8 token tiles per core
ND = D // 128        # 8 d chunks
PCOLS = 448          # 7 * R
CUM0 = 192           # start of cumsum group in P's columns
NCUM = 256           # cumsum group width

F32 = mybir.dt.float32
F16 = mybir.dt.float16


def build_nc():
    nc = bacc.Bacc(None, target_bir_lowering=False)

    xT = nc.dram_tensor("xT", [D, TH], F16, kind="ExternalInput")
    P = nc.dram_tensor("P", [D, PCOLS], F16, kind="ExternalInput")
    AT = nc.dram_tensor("AT", [128, D], F16, kind="ExternalInput")
    sxT = nc.dram_tensor("sxT", [128, ND], F16, kind="ExternalInput")
    invc = nc.dram_tensor("invc", [128, NT], F32, kind="ExternalInput")
    outT = nc.dram_tensor("outT", [D, TH], F16, kind="ExternalOutput")

    from concourse.masks import make_identity, make_upper_triangular

    with tile.TileContext(nc) as tc:
        with tc.tile_pool(name="consts", bufs=1) as consts, \
             tc.tile_pool(name="big", bufs=1) as big, \
             tc.tile_pool(name="outp", bufs=4) as outp, \
             tc.tile_pool(name="ps", bufs=2, space="PSUM") as ps:

            # ---- PE warmup: dependency-free matmul burst so the HAM
            # un-throttles (K=8/8) before the real stream arrives ----
            warm_sb = consts.tile([128, 128], F16)
            nc.vector.memset(warm_sb, 0.0)
            warm_ps = ps.tile([128, 512], F32, tag="work", bufs=6)
            for i in range(10):
                nc.tensor.matmul(warm_ps[:, 0:128], warm_sb, warm_sb,
                                 start=True, stop=True)

            # ---- constants built on idle engines (no DMA) ----
            U_sb = consts.tile([128, 128], F16)
            make_upper_triangular(nc, U_sb, val=1.0, diag=True)
            IDN_sb = consts.tile([128, 128], F16)
            make_identity(nc, IDN_sb)
            ones_sb = consts.tile([1, 128], F16)
            nc.vector.memset(ones_sb, 1.0)
            onescol_sb = consts.tile([128, 1], F16)
            nc.vector.memset(onescol_sb, 1.0)

            # ---- loads interleaved across both HWDGE queues so the
            # dk-chunk pairs arrive in consumption order ----
            xT_sb = big.tile([128, ND, TH], F16)
            P_sb = consts.tile([128, ND, PCOLS], F16)
            sxT_sb = consts.tile([128, ND], F16)
            invc_sb = consts.tile([128, NT], F32)
            AT_sb = consts.tile([128, D], F16)
            xTv = xT.rearrange("(k p) t -> p k t", p=128)
            Pv = P.rearrange("(k p) c -> p k c", p=128)
            for j in range(ND // 2):
                qx = nc.sync if j % 2 == 0 else nc.scalar
                qp = nc.scalar if j % 2 == 0 else nc.sync
                qx.dma_start(out=xT_sb[:, 2 * j:2 * j + 2, :],
                             in_=xTv[:, 2 * j:2 * j + 2, :])
                qp.dma_start(out=P_sb[:, 2 * j:2 * j + 2, :],
                             in_=Pv[:, 2 * j:2 * j + 2, :])
            nc.scalar.dma_start(out=sxT_sb, in_=sxT[:, :])
            nc.scalar.dma_start(out=invc_sb, in_=invc[:, :])
            nc.scalar.dma_start(out=AT_sb, in_=AT[:, :])

            xp_sb = big.tile([128, NT, PCOLS], F16)
            cum_sb = big.tile([128, NT, NCUM], F16)
            carry_sb = big.tile([1, NT, NCUM], F16)

            # ---- xp phase A: tiles 0..4 accumulate chunk-by-chunk right
            # behind the DMA stream (5 MMs/chunk ~ matches arrival rate) ----
            NA = 6
            gx = [ps.tile([128, PCOLS], F32, tag="work", bufs=6,
                          name=f"gx_{i}") for i in range(NA)]
            for dk in range(ND):
                for k in range(NA):
                    nc.tensor.matmul(gx[k],
                                     xT_sb[:, dk, k * 128:(k + 1) * 128],
                                     P_sb[:, dk, :],
                                     start=(dk == 0), stop=(dk == ND - 1))
            for k in range(NA):
                nc.vector.tensor_copy(xp_sb[:, k, CUM0:], gx[k][:, CUM0:])
                nc.scalar.copy(xp_sb[:, k, :CUM0], gx[k][:, :CUM0])

            def emit_xp(k):
                xp_ps = ps.tile([128, PCOLS], F32, tag="work", bufs=6)
                for dk in range(ND):
                    nc.tensor.matmul(xp_ps,
                                     xT_sb[:, dk, k * 128:(k + 1) * 128],
                                     P_sb[:, dk, :],
                                     start=(dk == 0), stop=(dk == ND - 1))
                nc.vector.tensor_copy(xp_sb[:, k, CUM0:], xp_ps[:, CUM0:])
                nc.scalar.copy(xp_sb[:, k, :CUM0], xp_ps[:, :CUM0])

            # carry-init: running total seeded with sx @ P[:, CUM0:]
            tot_ps = ps.tile([1, NCUM], F32, tag="tot", bufs=1)
            for dk in range(ND):
                nc.tensor.matmul(tot_ps, sxT_sb[:, dk:dk + 1],
                                 P_sb[:, dk, CUM0:],
                                 start=(dk == 0), stop=(dk == ND - 1))

            def emit_cum(k):
                # snapshot the carry (= totals of everything before tile k)
                nc.vector.tensor_copy(carry_sb[:, k, :], tot_ps[0:1, :])
                # append tile k's local column-total to the running total
                nc.tensor.matmul(tot_ps, onescol_sb, xp_sb[:, k, CUM0:],
                                 start=False, stop=True)
                # block cumsum + carry broadcast, then normalize by counts
                cum_ps = ps.tile([128, NCUM], F32, tag="cum", bufs=1)
                nc.tensor.matmul(cum_ps, U_sb, xp_sb[:, k, CUM0:],
                                 start=True, stop=False)
                nc.tensor.matmul(cum_ps, ones_sb, carry_sb[:, k, :],
                                 start=False, stop=True)
                nc.scalar.activation(cum_sb[:, k, :], cum_ps,
                                     mybir.ActivationFunctionType.Copy,
                                     scale=invc_sb[:, k:k + 1])

            # elementwise (tiles lo..hi): G = [g1|g2]
            G_sb = big.tile([128, NT, 128], F16)
            m2_sb = big.tile([128, NT, 64], F32)

            def emit_ew(lo, hi):
                xps = xp_sb[:, lo:hi, :]
                cms = cum_sb[:, lo:hi, :]
                m2 = m2_sb[:, lo:hi, :]
                g = G_sb[:, lo:hi, :]
                nc.vector.tensor_mul(m2, xps[:, :, 64:128], cms[:, :, 64:128])
                nc.vector.tensor_mul(g[:, :, 0:64], xps[:, :, 0:64],
                                     cms[:, :, 0:64])
                nc.vector.tensor_add(g[:, :, 0:64], g[:, :, 0:64], m2)
                nc.vector.tensor_mul(g[:, :, 64:128], xps[:, :, 128:192],
                                     cms[:, :, 128:192])
                nc.vector.tensor_mul(g[:, :, 64:128], g[:, :, 64:128],
                                     cms[:, :, 192:256])

            GT_sb = big.tile([128, TH], F16)

            def emit_tp(k):
                gt_ps = ps.tile([128, 128], F16, tag="work", bufs=6)
                nc.tensor.transpose(gt_ps, G_sb[:, k, :], IDN_sb)
                nc.vector.tensor_copy(GT_sb[:, k * 128:(k + 1) * 128], gt_ps)

            def emit_final(n):
                for dk in range(ND):
                    o_ps = ps.tile([128, 512], F32, tag="work", bufs=6)
                    nc.tensor.matmul(o_ps,
                                     AT_sb[:, dk * 128:(dk + 1) * 128],
                                     GT_sb[:, n * 512:(n + 1) * 512],
                                     start=True, stop=True)
                    o_sb = outp.tile([128, 512], F16)
                    nc.vector.tensor_copy(o_sb[:, 0:256], o_ps[:, 0:256])
                    nc.scalar.copy(o_sb[:, 256:512], o_ps[:, 256:512])
                    qo = nc.sync if dk % 2 == 0 else nc.scalar
                    qo.dma_start(
                        out=outT[dk * 128:(dk + 1) * 128,
                                 n * 512:(n + 1) * 512],
                        in_=o_sb)

            emit_xp(6)
            emit_cum(0)
            emit_cum(1)
            emit_xp(7)
            emit_cum(2)
            emit_cum(3)
            emit_ew(0, 4)
            for k in range(4):
                emit_tp(k)
            emit_final(0)
            emit_cum(4)
            emit_cum(5)
            emit_cum(6)
            emit_cum(7)
            emit_ew(4, NT)
            for k in range(4, NT):
                emit_tp(k)
            emit_final(1)

    nc.finalize()
    return nc


_NC = None


def _get_nc():
    global _NC
    if _NC is None:
        _NC = build_nc()
    return _NC


def _fold_weights(WQ, WK, WO, Winv, U_b, V_b, W_b, U_t, V_t, W_t, X_t,
                  alpha_bi, alpha_tri):
    f8 = np.float64
    WQ, WK, WO, Winv = (np.asarray(m) for m in (WQ, WK, WO, Winv))
    U_b, V_b, W_b = (np.asarray(m) for m in (U_b, V_b, W_b))
    U_t, V_t, W_t, X_t = (np.asarray(m) for m in (U_t, V_t, W_t, X_t))
    WQt = WQ.astype(f8).T
    WKt = WK.astype(f8).T
    Winvt = Winv.astype(f8).T
    P = np.concatenate([
        WQt @ V_b.astype(f8),
        float(alpha_bi) * (WQt @ (Winvt @ W_b.astype(f8))),
        WQt @ V_t.astype(f8),
        WKt @ W_b.astype(f8),
        WKt @ (Winvt @ V_b.astype(f8)),
        WKt @ W_t.astype(f8),
        X_t.astype(f8),
    ], axis=1).astype(np.float32)
    A = np.concatenate([
        WO.astype(f8) @ U_b.astype(f8),
        float(alpha_tri) * (WO.astype(f8) @ U_t.astype(f8)),
    ], axis=1).astype(np.float32)
    return P, A


def _make_consts(h):
    counts = np.arange(h * TH + 1, (h + 1) * TH + 1, dtype=np.float64)
    invc = np.ascontiguousarray(
        (1.0 / counts).astype(np.float32).reshape(NT, 128).T)
    return invc


def make_in_maps(x, P, A):
    AT = np.ascontiguousarray(A.T.astype(np.float16))
    P16 = P.astype(np.float16)
    in_maps = []
    for core in range(8):
        b, h = core // 2, core % 2
        xTc = np.ascontiguousarray(x[b, h * TH:(h + 1) * TH, :].T
                                   .astype(np.float16))
        if h == 1:
            sx = x[b, :TH, :].sum(axis=0, dtype=np.float64)
        else:
            sx = np.zeros(D, np.float64)
        sxT = np.ascontiguousarray(
            sx.astype(np.float16).reshape(ND, 128).T)
        invc = _make_consts(h)
        in_maps.append(dict(xT=xTc, P=P16, AT=AT, sxT=sxT, invc=invc))
    return in_maps


def kernel(x, WQ, WK, WO, Winv, U_b, V_b, W_b, bias_b,
           U_t, V_t, W_t, X_t, bias_t, alpha_bi, alpha_tri):
    x = np.asarray(x, dtype=np.float32)
    P, A = _fold_weights(WQ, WK, WO, Winv, U_b, V_b, W_b,
                         U_t, V_t, W_t, X_t, alpha_bi, alpha_tri)
    in_maps = make_in_maps(x, P, A)

    res = run_bass_kernel_spmd(_get_nc(), in_maps, core_ids=list(range(8)))

    out = np.empty((B, T, D), np.float32)
    for core in range(8):
        b, h = core // 2, core % 2
        out[b, h * TH:(h + 1) * TH, :] = \
            res.results[core]["outT"].T.astype(np.float32)

    # constant bias term (zero for the given inputs, kept for fidelity)
    bias_out = ((1.0 + float(alpha_bi)) * np.asarray(bias_b, np.float64)
                + float(alpha_tri) * np.asarray(bias_t, np.float64)) \
        @ np.asarray(WO, np.float64).T
    if np.any(bias_out):
        out += bias_out.astype(np.float32)[None, None, :]
    return out



# revision 3
# speedup vs baseline: 1.1709x; 1.1709x over previous
"""CausalBiTrilinearBCNAttention Trainium2 kernel (layout-B rewrite).

Math: the network collapses to xp = x @ P (448 rank columns), causal
cumsums over 4 of the 7 rank groups, elementwise rank products, and a
final [T,128]@[128,D] projection (see P/A folding below).

This version keeps FEATURES on partitions and TOKENS on the free dim:

  xpT = P.T @ x.T        4 stationary groups of <=128 P-columns,
                         tokens stream as the moving operand
  cums = tensor_tensor_scan (DVE prefix-add along free dim, fp32 state)
  ew   = lane-aligned DVE products (P column order is chosen so every
         product pairs values living on the same partitions):
           P cols = [b3|b1 | b7|b2 | a3|a1 | a2s]
           C0 = scan(G3) = [c3@lo | c1@hi]   C0p = C0 * invc
           C1 = scan(G4) = [c7@lo | c2@hi]   C1p = C1 * invc
           g2 = a3 * (c3'*c7')  @lo          -> GT[0:64]
           g1 = a1*c1' + a2s*c2'  @hi        -> GT[64:128]
  out  = A'.T @ GT       A' = [alpha_tri*WO@U_t | WO@U_b]

so there are no PE transposes, no PE cumsum/carry chain, and the PE
stream is 64 xpT MMs + 16 final MMs, all N=512, back to back (HAM
stays warm; a dummy-MM burst warms it during the DMA lead-in).

Sharding: 8 cores = 4 batches x 2 T-halves. The T/2 carry for the
second half is folded by the host into the scan initial values
(carry = sum_t x[b,:T/2] @ P, fp32) and chained across the two
512-token device halves via the scan output's last column.
"""

import numpy as np

import concourse.bass as bass
import concourse.tile as tile
from concourse import bacc, mybir
from concourse.bass_utils import run_bass_kernel_spmd

B, T, D, R = 4, 2048, 1024, 64
TH = T // 2          # tokens per core
ND = D // 128        # 8 contraction chunks
PCOLS = 448
HW = 512             # tokens per device half
F32 = mybir.dt.float32
F16 = mybir.dt.float16

# P' column groups (each <=128 wide -> one stationary tile):
#   G3  = cols   0:128 = [b3|b1]   (cumsummed)
#   G4  = cols 128:256 = [b7|b2]   (cumsummed)
#   A1  = cols 256:384 = [a3|a1]
#   A2s = cols 384:448 = [a2s]     (M=64, written to partitions 64:128)
G3_, G4_, A1_, A2s_ = (0, 128), (128, 256), (256, 384), (384, 448)

ADD = mybir.AluOpType.add
BYP = mybir.AluOpType.bypass


def build_nc():
    nc = bacc.Bacc(None, target_bir_lowering=False)

    xT = nc.dram_tensor("xT", [D, TH], F16, kind="ExternalInput")
    P = nc.dram_tensor("P", [D, PCOLS], F16, kind="ExternalInput")
    AT = nc.dram_tensor("AT", [128, D], F16, kind="ExternalInput")
    invcT = nc.dram_tensor("invcT", [128, TH], F16, kind="ExternalInput")
    carry = nc.dram_tensor("carry", [128, 2], F32, kind="ExternalInput")
    outT = nc.dram_tensor("outT", [D, TH], F16, kind="ExternalOutput")

    xv = xT.rearrange("(k p) t -> p k t", p=128)
    Pv = P.rearrange("(k p) c -> p k c", p=128)
    ov = outT.rearrange("(k p) t -> p k t", p=128)

    with tile.TileContext(nc) as tc:
        with tc.tile_pool(name="consts", bufs=1) as consts, \
             tc.tile_pool(name="big", bufs=1) as big, \
             tc.tile_pool(name="outp", bufs=4) as outp, \
             tc.tile_pool(name="ps", bufs=1, space="PSUM") as ps:

            # ---- HAM warmup: dependency-light dummy MMs so the PE activity
            # window is already "busy" when the real stream arrives ----
            warm_sb = consts.tile([128, 128], F16)
            nc.gpsimd.memset(warm_sb, 0.0)
            warm_ps = ps.tile([128, 512], F32, tag="out", bufs=3)
            for _ in range(10):
                nc.tensor.matmul(warm_ps[:, 0:128], warm_sb, warm_sb,
                                 start=True, stop=True)

            # ---- SBUF tiles ----
            xT_sb = big.tile([128, ND, TH], F16)
            P_sb = consts.tile([128, ND, PCOLS], F16)
            AT_sb = consts.tile([128, D], F16)
            invcT_sb = consts.tile([128, TH], F16)
            carry_sb = consts.tile([128, 2], F32)

            C0raw = big.tile([128, 2, HW], F16)
            C1raw = big.tile([128, 2, HW], F16)
            C0p = big.tile([128, 2, HW], F16)
            C1p = big.tile([128, 2, HW], F16)
            A1sb = big.tile([128, 2, HW], F16)
            A2sb = big.tile([128, 2, HW], F16)
            t2sb = big.tile([64, 2, HW], F16)
            m1sb = big.tile([128, 2, HW], F16)
            m2sb = big.tile([128, 2, HW], F16)
            GT = big.tile([128, 2, HW], F16)

            # ---- input DMAs, interleaved for just-in-time arrival of the
            # dk-major half-0 sweep, then half-1 + AT ----
            nc.sync.dma_start(out=xT_sb[:, 0, 0:HW], in_=xv[:, 0, 0:HW])
            nc.sync.dma_start(out=P_sb[:, 0:4, :], in_=Pv[:, 0:4, :])
            nc.sync.dma_start(out=xT_sb[:, 3, 0:HW], in_=xv[:, 3, 0:HW])
            nc.sync.dma_start(out=xT_sb[:, 5, 0:HW], in_=xv[:, 5, 0:HW])
            nc.sync.dma_start(out=xT_sb[:, 7, 0:HW], in_=xv[:, 7, 0:HW])
            nc.sync.dma_start(out=xT_sb[:, 0:4, HW:TH], in_=xv[:, 0:4, HW:TH])

            nc.scalar.dma_start(out=carry_sb, in_=carry[:, :])
            nc.scalar.dma_start(out=xT_sb[:, 1, 0:HW], in_=xv[:, 1, 0:HW])
            nc.scalar.dma_start(out=xT_sb[:, 2, 0:HW], in_=xv[:, 2, 0:HW])
            nc.scalar.dma_start(out=P_sb[:, 4:8, :], in_=Pv[:, 4:8, :])
            nc.scalar.dma_start(out=xT_sb[:, 4, 0:HW], in_=xv[:, 4, 0:HW])
            nc.scalar.dma_start(out=xT_sb[:, 6, 0:HW], in_=xv[:, 6, 0:HW])
            nc.scalar.dma_start(out=xT_sb[:, 4:6, HW:TH], in_=xv[:, 4:6, HW:TH])
            nc.scalar.dma_start(out=invcT_sb, in_=invcT[:, :])
            nc.scalar.dma_start(out=xT_sb[:, 6:8, HW:TH], in_=xv[:, 6:8, HW:TH])
            nc.scalar.dma_start(out=AT_sb, in_=AT[:, :])

            def emit_scans(h, g3p, g4p):
                init0 = carry_sb[:, 0:1] if h == 0 else C0raw[:, 0, HW - 1:HW]
                init1 = carry_sb[:, 1:2] if h == 0 else C1raw[:, 0, HW - 1:HW]
                iv = invcT_sb[:, h * HW:(h + 1) * HW]
                nc.vector.tensor_tensor_scan(C0raw[:, h, :], g3p[0:128, :], iv,
                                             init0, ADD, BYP)
                nc.vector.tensor_tensor_scan(C1raw[:, h, :], g4p[0:128, :], iv,
                                             init1, ADD, BYP)
                nc.vector.tensor_mul(C0p[:, h, :], C0raw[:, h, :], iv)
                nc.vector.tensor_mul(C1p[:, h, :], C1raw[:, h, :], iv)

            def emit_ew(h):
                # lane-aligned products; GT rows 0:64 = g2(tri), 64:128 = g1
                nc.vector.tensor_mul(t2sb[:, h, :], C0p[0:64, h, :],
                                     C1p[0:64, h, :])
                nc.vector.tensor_mul(GT[0:64, h, :], A1sb[0:64, h, :],
                                     t2sb[:, h, :])
                nc.vector.tensor_mul(m1sb[64:128, h, :], A1sb[64:128, h, :],
                                     C0p[64:128, h, :])
                nc.vector.tensor_mul(m2sb[64:128, h, :], A2sb[64:128, h, :],
                                     C1p[64:128, h, :])
                nc.vector.tensor_add(GT[64:128, h, :], m1sb[64:128, h, :],
                                     m2sb[64:128, h, :])

            def emit_final(h, qflip):
                for dk in range(ND):
                    o_ps = ps.tile([128, 512], F32, tag="out", bufs=3)
                    nc.tensor.matmul(o_ps, AT_sb[:, dk * 128:(dk + 1) * 128],
                                     GT[:, h, :], start=True, stop=True)
                    if dk % 2 == 0:
                        osb = outp.tile([128, 2, HW], F16)
                        nc.vector.tensor_copy(osb[:, 0, :], o_ps)
                    else:
                        nc.scalar.copy(osb[:, 1, :], o_ps)
                        q = nc.sync if (dk // 2 + qflip) % 2 == 0 else nc.scalar
                        q.dma_start(out=ov[:, dk - 1:dk + 1,
                                           h * HW:(h + 1) * HW],
                                    in_=osb)

            # ================= half 0: dk-major (DMA-paced) =================
            h = 0
            g3p = ps.tile([128, HW], F32, tag="g3", bufs=1)
            g4p = ps.tile([128, HW], F32, tag="g4", bufs=1)
            a1p = ps.tile([128, HW], F32, tag="a1", bufs=2)
            a2p = ps.tile([128, HW], F32, tag="a2s", bufs=1)
            for dk in range(ND):
                st, sp = (dk == 0), (dk == ND - 1)
                xh = xT_sb[:, dk, 0:HW]
                nc.tensor.matmul(g3p, P_sb[:, dk, 0:128], xh, start=st, stop=sp)
                nc.tensor.matmul(g4p, P_sb[:, dk, 128:256], xh, start=st, stop=sp)
                nc.tensor.matmul(a1p, P_sb[:, dk, 256:384], xh, start=st, stop=sp)
                nc.tensor.matmul(a2p[64:128, :], P_sb[:, dk, 384:448], xh,
                                 start=st, stop=sp)
            nc.scalar.copy(A1sb[:, 0, :], a1p)
            nc.scalar.copy(A2sb[64:128, 0, :], a2p[64:128, :])
            emit_scans(0, g3p, g4p)
            emit_ew(0)

            # ================= half 1 groups + interleaved h0 final =========
            h = 1
            a1p1 = ps.tile([128, HW], F32, tag="a1", bufs=2)
            for dk in range(ND):
                nc.tensor.matmul(a1p1, P_sb[:, dk, 256:384],
                                 xT_sb[:, dk, HW:TH],
                                 start=(dk == 0), stop=(dk == ND - 1))
            nc.scalar.copy(A1sb[:, 1, :], a1p1)

            g3p1 = ps.tile([128, HW], F32, tag="g3", bufs=1)
            for dk in range(ND):
                nc.tensor.matmul(g3p1, P_sb[:, dk, 0:128],
                                 xT_sb[:, dk, HW:TH],
                                 start=(dk == 0), stop=(dk == ND - 1))

            emit_final(0, 0)

            g4p1 = ps.tile([128, HW], F32, tag="g4", bufs=1)
            for dk in range(ND):
                nc.tensor.matmul(g4p1, P_sb[:, dk, 128:256],
                                 xT_sb[:, dk, HW:TH],
                                 start=(dk == 0), stop=(dk == ND - 1))
            emit_scans(1, g3p1, g4p1)

            a2p1 = ps.tile([128, HW], F32, tag="a2s", bufs=1)
            for dk in range(ND):
                nc.tensor.matmul(a2p1[64:128, :], P_sb[:, dk, 384:448],
                                 xT_sb[:, dk, HW:TH],
                                 start=(dk == 0), stop=(dk == ND - 1))
            nc.scalar.copy(A2sb[64:128, 1, :], a2p1[64:128, :])
            emit_ew(1)

            emit_final(1, 0)

    nc.finalize()
    return nc


_NC = None


def _get_nc():
    global _NC
    if _NC is None:
        _NC = build_nc()
    return _NC


def _fold_weights(WQ, WK, WO, Winv, U_b, V_b, W_b, U_t, V_t, W_t, X_t,
                  alpha_bi, alpha_tri):
    f8 = np.float64
    WQ, WK, WO, Winv = (np.asarray(m) for m in (WQ, WK, WO, Winv))
    U_b, V_b, W_b = (np.asarray(m) for m in (U_b, V_b, W_b))
    U_t, V_t, W_t, X_t = (np.asarray(m) for m in (U_t, V_t, W_t, X_t))
    WQt = WQ.astype(f8).T
    WKt = WK.astype(f8).T
    Winvt = Winv.astype(f8).T
    # P' columns: [b3 | b1 | b7 | b2 | a3 | a1 | a2s]
    P = np.concatenate([
        WKt @ W_t.astype(f8),                              # b3
        WKt @ W_b.astype(f8),                              # b1
        X_t.astype(f8),                                    # b7
        WKt @ (Winvt @ V_b.astype(f8)),                    # b2
        WQt @ V_t.astype(f8),                              # a3
        WQt @ V_b.astype(f8),                              # a1
        float(alpha_bi) * (WQt @ (Winvt @ W_b.astype(f8))),  # a2s
    ], axis=1)
    # A' columns: [alpha_tri*WO@U_t | WO@U_b]  (GT rows: g2 then g1)
    A = np.concatenate([
        float(alpha_tri) * (WO.astype(f8) @ U_t.astype(f8)),
        WO.astype(f8) @ U_b.astype(f8),
    ], axis=1)
    return P, A


def make_in_maps(x, P, A):
    AT = np.ascontiguousarray(A.T.astype(np.float16))
    P16 = np.ascontiguousarray(P.astype(np.float16))
    in_maps = []
    for core in range(8):
        b, h = core // 2, core % 2
        xTc = np.ascontiguousarray(x[b, h * TH:(h + 1) * TH, :].T
                                   .astype(np.float16))
        if h == 1:
            sxP = x[b, :TH, :].astype(np.float64).sum(axis=0) @ P
            carry = np.stack([sxP[0:128], sxP[128:256]], axis=1)
        else:
            carry = np.zeros((128, 2), np.float64)
        counts = np.arange(h * TH + 1, (h + 1) * TH + 1, dtype=np.float64)
        invcT = np.broadcast_to((1.0 / counts).astype(np.float16),
                                (128, TH))
        in_maps.append(dict(xT=xTc, P=P16, AT=AT,
                            invcT=np.ascontiguousarray(invcT),
                            carry=np.ascontiguousarray(
                                carry.astype(np.float32))))
    return in_maps


def kernel(x, WQ, WK, WO, Winv, U_b, V_b, W_b, bias_b,
           U_t, V_t, W_t, X_t, bias_t, alpha_bi, alpha_tri):
    x = np.asarray(x, dtype=np.float32)
    P, A = _fold_weights(WQ, WK, WO, Winv, U_b, V_b, W_b,
                         U_t, V_t, W_t, X_t, alpha_bi, alpha_tri)
    in_maps = make_in_maps(x, P, A)

    res = run_bass_kernel_spmd(_get_nc(), in_maps, core_ids=list(range(8)))

    out = np.empty((B, T, D), np.float32)
    for core in range(8):
        b, h = core // 2, core % 2
        out[b, h * TH:(h + 1) * TH, :] = \
            res.results[core]["outT"].T.astype(np.float32)

    # constant bias term (zero for the given inputs, kept for fidelity)
    bias_out = ((1.0 + float(alpha_bi)) * np.asarray(bias_b, np.float64)
                + float(alpha_tri) * np.asarray(bias_t, np.float64)) \
        @ np.asarray(WO, np.float64).T
    if np.any(bias_out):
        out += bias_out.astype(np.float32)[None, None, :]
    return out
